# revision 2
# baseline (speedup 1.0000x reference)
"""BalanceLoss Trainium2 kernel.

Math restructuring (see reference _balance_loss):
  - pos_gt = (pos_sum >= B/2), neg_gt = (pos_sum < B/2) are complementary, so
    maj/min groups partition the batch and maj_cnt/min_cnt derive from pos_sum.
  - easy(t=1) <=> x > ln2 <=> sp(-x) < tau ; easy(t=0) <=> x < -ln2 <=>
    sp(x) < tau, with tau = ln(1.5) = softplus(-ln2).
  - loss needs 5 per-column sums over the batch:
        pos   = sum t
        S1    = sum_{t=1} sp(-x)        S1h = sum_{t=1, hard} sp(-x)
        S0    = sum_{t=0} sp(x)         S0h = sum_{t=0, hard} sp(x)
    per column:  maj = positives if pos_gt else negatives
        loss_c = maj_scale*Shard_maj + min_scale*S_min ;  total /(B*C)

Device (per core, data-parallel over batch), bf16 inputs x=pred, tm1=t-1:
  ACT: ex = exp(x) ; v = ln(ex + 1) = softplus(x)        (one LUT table)
  DVE: vn = v - x (= softplus(-x)) ; a1 = (tm1+1)*vn ; h1 = [vn>=tau]*a1
       a0 = tm1*v ; h0 = [v>=tau]*a0     (all-bf16 SBUF ops -> fast DVE mode)
  Pool: pairwise-fold tm1/a1/a0 to halve their matmul rows
  PE:  ones[128,1]^T @ {tm1f, a1f, h1, a0, h0} -> 5 PSUM accumulators
Host: pos = B + sum(tm1); S1 = sum a1; S1h = sum h1; S0 = -sum a0;
      S0h = -sum h0; then scales/means (tiny, per-column only).
"""

import numpy as np

B_TOTAL = 131072
C = 128
N_CORES = 8
ROWS = B_TOTAL // N_CORES      # 16384 rows per core
FD = 2048                      # free-dim elements per chunk tile
J = FD // C                    # rows folded per partition per chunk
N_CHUNKS = ROWS * C // (128 * FD)   # 8
MM_N = 512                     # matmul moving free dim (one PSUM bank)
N_STREAMS = 5
TAU = float(np.log(1.5))       # softplus(-ln2): easy/hard boundary

_CACHE = {}


def _pin_act_tables():
    """Force the single LUT set containing both exp and ln so the kernel
    loads one ACT table instead of ping-ponging between two (1.3us/reload).
    Set indices must keep matching act_info.json, so empty the others."""
    import concourse.bacc as bacc
    import concourse.hw_specs as hw_specs

    if getattr(hw_specs, "_act_tables_pinned", False):
        return
    orig = hw_specs.get_activation_tables

    def patched(arch):
        tabs = dict(orig(arch))
        keep = "natural_log_exp_and_others"
        if keep in tabs:
            tabs = {n: (s if n == keep else set()) for n, s in tabs.items()}
        return tabs

    hw_specs._act_tables_pinned = True
    hw_specs.get_activation_tables = patched
    bacc.get_activation_tables = patched


def _build_nc():
    import concourse.bacc as bacc
    import concourse.tile as tile
    from concourse import mybir

    _pin_act_tables()

    f32 = mybir.dt.float32
    bf16 = mybir.dt.bfloat16
    AF = mybir.ActivationFunctionType
    OP = mybir.AluOpType

    nc = bacc.Bacc(None)
    xd = nc.dram_tensor("x", [ROWS, C], bf16, kind="ExternalInput")
    td = nc.dram_tensor("tm1", [ROWS, C], bf16, kind="ExternalInput")
    out = nc.dram_tensor("partials", [1, N_STREAMS * MM_N], f32,
                         kind="ExternalOutput")

    # row = m*FD + p*J + j ; per-partition contiguous J*C elements
    x_r = xd.rearrange("(m p j) c -> m p (j c)", p=128, j=J)
    t_r = td.rearrange("(m p j) c -> m p (j c)", p=128, j=J)

    FD2 = FD // 2
    n_sub = FD // MM_N
    with tile.TileContext(nc) as tc:
        with (
            tc.tile_pool(name="singles", bufs=1) as singles,
            tc.tile_pool(name="io", bufs=4) as io,
            tc.tile_pool(name="work", bufs=2) as work,
            tc.tile_pool(name="psum", bufs=1, space="PSUM") as psum_pool,
        ):
            ones = singles.tile([128, 1], bf16)
            nc.vector.memset(ones, 1.0)
            acc = [
                psum_pool.tile([1, MM_N], f32, tag=f"acc{s}", name=f"acc{s}")
                for s in range(N_STREAMS)
            ]
            # Warmup matmul consumes the ones-memset dependency so that
            # steady-state matmuls carry at most one sync wait (walrus
            # LDWEIGHTS codegen supports only one).
            warm = psum_pool.tile([1, 1], f32, tag="warm")
            nc.tensor.matmul(warm, ones, ones, start=True, stop=True)
            for m in range(N_CHUNKS):
                x = io.tile([128, FD], bf16, tag="x")
                t = io.tile([128, FD], bf16, tag="t")
                nc.sync.dma_start(x, x_r[m])
                nc.sync.dma_start(t, t_r[m])

                ex = work.tile([128, FD], bf16, tag="ex")
                v = work.tile([128, FD], bf16, tag="v")
                nc.scalar.activation(ex, x, AF.Exp)
                nc.scalar.activation(v, ex, AF.Ln, bias=1.0)

                vn = work.tile([128, FD], bf16, tag="vn")
                a1 = work.tile([128, FD], bf16, tag="a1")
                h1 = work.tile([128, FD], bf16, tag="h1")
                a0 = work.tile([128, FD], bf16, tag="a0")
                h0 = work.tile([128, FD], bf16, tag="h0")
                # vn = softplus(-x) = v - x
                nc.vector.scalar_tensor_tensor(
                    vn, v, 0.0, x, OP.bypass, OP.subtract)
                # a1 = t*vn ; h1 = [vn>=tau]*a1 (hard positives)
                nc.vector.scalar_tensor_tensor(
                    a1, t, 1.0, vn, OP.add, OP.mult)
                nc.vector.scalar_tensor_tensor(
                    h1, vn, TAU, a1, OP.is_ge, OP.mult)
                # a0 = (t-1)*v = -[t==0]*v ; h0 = [v>=tau]*a0 (-hard negatives)
                nc.vector.tensor_tensor(a0, t, v, OP.mult)
                nc.vector.scalar_tensor_tensor(
                    h0, v, TAU, a0, OP.is_ge, OP.mult)

                # pairwise fold tm1/a1/a0 on Pool to halve their PE rows
                tf = work.tile([128, FD2], bf16, tag="tf")
                a1f = work.tile([128, FD2], bf16, tag="a1f")
                a0f = work.tile([128, FD2], bf16, tag="a0f")
                nc.gpsimd.tensor_tensor(tf, t[:, 0:FD2], t[:, FD2:FD], OP.add)
                nc.gpsimd.tensor_tensor(a1f, a1[:, 0:FD2], a1[:, FD2:FD], OP.add)
                nc.gpsimd.tensor_tensor(a0f, a0[:, 0:FD2], a0[:, FD2:FD], OP.add)

                movers = [(tf, FD2), (a1f, FD2), (h1, FD), (a0f, FD2), (h0, FD)]
                for s, (mv, w) in enumerate(movers):
                    for jj in range(w // MM_N):
                        nc.tensor.matmul(
                            acc[s][:, :],
                            ones[:, :],
                            mv[:, jj * MM_N : (jj + 1) * MM_N],
                            start=(m == 0 and jj == 0),
                            stop=(m == N_CHUNKS - 1 and jj == w // MM_N - 1),
                        )

            res = singles.tile([1, N_STREAMS * MM_N], f32)
            for s in range(N_STREAMS):
                nc.vector.tensor_copy(
                    res[:, s * MM_N : (s + 1) * MM_N], acc[s][:, :])
            nc.sync.dma_start(out[:, :], res)
    nc.finalize()
    return nc


def _get_nc():
    if "nc" not in _CACHE:
        _CACHE["nc"] = _build_nc()
    return _CACHE["nc"]


def _in_maps(pred, target):
    import ml_dtypes

    bf = ml_dtypes.bfloat16
    x = np.asarray(pred, dtype=np.float32).astype(bf)
    tm1 = (np.asarray(target, dtype=np.float32) - 1.0).astype(bf)
    return [
        {
            "x": np.ascontiguousarray(x[i * ROWS : (i + 1) * ROWS]),
            "tm1": np.ascontiguousarray(tm1[i * ROWS : (i + 1) * ROWS]),
        }
        for i in range(N_CORES)
    ]


def _combine(parts):
    """parts: [n_cores, 5, MM_N] raw psum rows -> final scalar loss."""
    # psum col q sums j-groups with (j % (MM_N//C)) == q//C at col q % C;
    # fold the leftover j-groups and cores.
    S = parts.reshape(-1, N_STREAMS, MM_N // C, C).sum(axis=(0, 2),
                                                       dtype=np.float64)
    st, s1, s1h, na0, nh0 = S
    B = float(B_TOTAL)
    pos = B + st
    s0, s0h = -na0, -nh0
    bal = 0.5 * B
    pos_gt = pos >= bal
    maj_cnt = np.where(pos_gt, pos, B - pos)
    min_cnt = B - maj_cnt
    maj_scale = bal / np.maximum(maj_cnt, 1.0)
    min_scale = np.where(min_cnt > 0, (B - bal) / np.maximum(min_cnt, 1.0), 1.0)
    s_maj_hard = np.where(pos_gt, s1h, s0h)
    s_min = np.where(pos_gt, s0, s1)
    total = (maj_scale * s_maj_hard + min_scale * s_min).sum()
    return np.float32(total / (B * C))


def kernel(pred: np.ndarray, target: np.ndarray) -> np.ndarray:
    from concourse.bass_utils import run_bass_kernel_spmd

    nc = _get_nc()
    res = run_bass_kernel_spmd(
        nc, _in_maps(pred, target), core_ids=list(range(N_CORES)))
    parts = np.stack(
        [r["partials"].reshape(N_STREAMS, MM_N) for r in res.results])
    return _combine(parts)


# revision 4
# speedup vs baseline: 2.0747x; 2.0747x over previous
"""BalanceLoss Trainium2 kernel.

Math restructuring (see reference _balance_loss):
  - pos_gt = (pos_sum >= B/2) and neg_gt are complementary, so maj/min
    groups partition the batch and their counts derive from pos_sum.
  - With y = (1-2t)*x (sign-folded logits, an input encoding choice):
      per-element BCE  = softplus(-x) + (1-t)*x = softplus(y)  exactly,
      easy <=> g < 1/3 <=> softplus(y) < tau,  tau = ln(1.5),
    so ONE softplus and ONE threshold serve both classes.
  - loss needs 5 per-column sums over the batch:
        pos = sum t          T   = sum v         Th  = sum_{hard} v
        S1  = sum_{t=1} v    S1h = sum_{t=1, hard} v      (v = softplus(y))
    then S0 = T - S1, S0h = Th - S1h, and per column:
        loss_c = maj_scale * Shard_maj + min_scale * S_min ;  total /(B*C)

Device (per core, data-parallel over batch), bf16 inputs y, t:
  ACT: q = exp(y) ; v = ln(q + 1) = softplus(y)      (one LUT table)
  DVE: m = [v >= tau] ; hv = m*v ; a1 = t*v ; h1 = t*hv
       (tensor_tensor / tensor_scalar only: these hit the DVE 2x perf mode;
        scalar_tensor_tensor measures 1x so it is avoided)
  Pool: computes the tail half of a1 (the one dependency-free product) to
        keep DVE at ~3.5 passes/chunk, below the ACT wall.
  PE:  ones[128,1]^T @ {t, v, hv, a1, h1} -> 5 PSUM accumulators
Host: unshard + tiny per-column combine (scales, mean).
"""

import numpy as np

B_TOTAL = 131072
C = 128
N_CORES = 8
ROWS = B_TOTAL // N_CORES      # 16384 rows per core
FD = 2048                      # free-dim elements per chunk tile
J = FD // C                    # rows folded per partition per chunk
N_CHUNKS = ROWS * C // (128 * FD)   # 8
MM_N = 512                     # matmul moving free dim (one PSUM bank)
N_STREAMS = 5
SPLIT = 1024                   # a1 columns [SPLIT:] computed on Pool
TAU = float(np.log(1.5))       # softplus(-ln2): easy/hard boundary

_CACHE = {}


def _pin_act_tables():
    """Force the single LUT set containing both exp and ln so the kernel
    loads one ACT table instead of ping-ponging between two (1.3us/reload).
    Set indices must keep matching act_info.json, so empty the others."""
    import concourse.bacc as bacc
    import concourse.hw_specs as hw_specs

    if getattr(hw_specs, "_act_tables_pinned", False):
        return
    orig = hw_specs.get_activation_tables

    def patched(arch):
        tabs = dict(orig(arch))
        keep = "natural_log_exp_and_others"
        if keep in tabs:
            tabs = {n: (s if n == keep else set()) for n, s in tabs.items()}
        return tabs

    hw_specs._act_tables_pinned = True
    hw_specs.get_activation_tables = patched
    bacc.get_activation_tables = patched


def _build_nc():
    import concourse.bacc as bacc
    import concourse.tile as tile
    from concourse import mybir

    _pin_act_tables()

    f32 = mybir.dt.float32
    bf16 = mybir.dt.bfloat16
    AF = mybir.ActivationFunctionType
    OP = mybir.AluOpType

    nc = bacc.Bacc(None)
    yd = nc.dram_tensor("y", [ROWS, C], bf16, kind="ExternalInput")
    td = nc.dram_tensor("t", [ROWS, C], bf16, kind="ExternalInput")
    out = nc.dram_tensor("partials", [1, N_STREAMS * MM_N], f32,
                         kind="ExternalOutput")

    # row = m*FD + p*J + j ; per-partition contiguous J*C elements
    y_r = yd.rearrange("(m p j) c -> m p (j c)", p=128, j=J)
    t_r = td.rearrange("(m p j) c -> m p (j c)", p=128, j=J)

    with tile.TileContext(nc) as tc:
        with (
            tc.tile_pool(name="singles", bufs=1) as singles,
            tc.tile_pool(name="io", bufs=4) as io,
            tc.tile_pool(name="work", bufs=2) as work,
            tc.tile_pool(name="psum", bufs=1, space="PSUM") as psum_pool,
        ):
            ones = singles.tile([128, 1], bf16)
            nc.vector.memset(ones, 1.0)
            acc = psum_pool.tile([1, N_STREAMS * MM_N], f32, tag="acc")
            # Warmup matmul consumes the ones-memset dependency so that
            # steady-state matmuls carry at most one sync wait (walrus
            # LDWEIGHTS codegen supports only one).
            warm = psum_pool.tile([1, 1], f32, tag="warm")
            nc.tensor.matmul(warm, ones, ones, start=True, stop=True)

            def mm(s, mv, lo, hi, m):
                first = m == 0
                last = m == N_CHUNKS - 1
                for jj in range(lo // MM_N, hi // MM_N):
                    nc.tensor.matmul(
                        acc[:, s * MM_N : (s + 1) * MM_N],
                        ones[:, :],
                        mv[:, jj * MM_N : (jj + 1) * MM_N],
                        start=(first and jj == lo // MM_N),
                        stop=(last and jj == hi // MM_N - 1),
                    )

            for m in range(N_CHUNKS):
                y = io.tile([128, FD], bf16, tag="y")
                t = io.tile([128, FD], bf16, tag="t")
                nc.sync.dma_start(y, y_r[m])
                nc.sync.dma_start(t, t_r[m])
                mm(0, t, 0, FD, m)

                q = work.tile([128, FD], bf16, tag="q")
                v = work.tile([128, FD], bf16, tag="v")
                nc.scalar.activation(q, y, AF.Exp)
                nc.scalar.activation(v, q, AF.Ln, bias=1.0)
                mm(1, v, 0, FD, m)

                msk = work.tile([128, FD], bf16, tag="msk")
                hv = work.tile([128, FD], bf16, tag="hv")
                a1 = work.tile([128, FD], bf16, tag="a1")
                h1 = work.tile([128, FD], bf16, tag="h1")
                # a1 = t*v split across DVE ([:SPLIT]) and Pool ([SPLIT:])
                nc.vector.tensor_tensor(
                    a1[:, 0:SPLIT], t[:, 0:SPLIT], v[:, 0:SPLIT], OP.mult)
                nc.gpsimd.tensor_tensor(
                    a1[:, SPLIT:FD], t[:, SPLIT:FD], v[:, SPLIT:FD], OP.mult)
                mm(3, a1, 0, FD, m)
                # hard mask and hard-masked values
                nc.vector.tensor_scalar(msk, v, TAU, None, OP.is_ge)
                nc.vector.tensor_tensor(hv, msk, v, OP.mult)
                mm(2, hv, 0, FD, m)
                nc.vector.tensor_tensor(h1, t, hv, OP.mult)
                mm(4, h1, 0, FD, m)

            res = singles.tile([1, N_STREAMS * MM_N], f32)
            nc.vector.tensor_copy(res, acc)
            nc.sync.dma_start(out[:, :], res)
    nc.finalize()
    return nc


def _get_nc():
    if "nc" not in _CACHE:
        _CACHE["nc"] = _build_nc()
    return _CACHE["nc"]


def _in_maps(pred, target):
    import ml_dtypes

    bf = ml_dtypes.bfloat16
    p32 = np.asarray(pred, dtype=np.float32)
    t32 = np.asarray(target, dtype=np.float32)
    y = ((1.0 - 2.0 * t32) * p32).astype(bf)   # exact sign flip of pred
    t = t32.astype(bf)
    return [
        {
            "y": np.ascontiguousarray(y[i * ROWS : (i + 1) * ROWS]),
            "t": np.ascontiguousarray(t[i * ROWS : (i + 1) * ROWS]),
        }
        for i in range(N_CORES)
    ]


def _combine(parts):
    """parts: [n_cores, 5, MM_N] raw psum rows -> final scalar loss."""
    # psum col q sums j-groups with (j % (MM_N//C)) == q//C at col q % C;
    # fold the leftover j-groups and cores.
    S = parts.reshape(-1, N_STREAMS, MM_N // C, C).sum(axis=(0, 2),
                                                       dtype=np.float64)
    pos, T, Th, s1, s1h = S
    B = float(B_TOTAL)
    s0, s0h = T - s1, Th - s1h
    bal = 0.5 * B
    pos_gt = pos >= bal
    maj_cnt = np.where(pos_gt, pos, B - pos)
    min_cnt = B - maj_cnt
    maj_scale = bal / np.maximum(maj_cnt, 1.0)
    min_scale = np.where(min_cnt > 0, (B - bal) / np.maximum(min_cnt, 1.0), 1.0)
    s_maj_hard = np.where(pos_gt, s1h, s0h)
    s_min = np.where(pos_gt, s0, s1)
    total = (maj_scale * s_maj_hard + min_scale * s_min).sum()
    return np.float32(total / (B * C))


def kernel(pred: np.ndarray, target: np.ndarray) -> np.ndarray:
    from concourse.bass_utils import run_bass_kernel_spmd

    nc = _get_nc()
    res = run_bass_kernel_spmd(
        nc, _in_maps(pred, target), core_ids=list(range(N_CORES)))
    parts = np.stack(
        [r["partials"].reshape(N_STREAMS, MM_N) for r in res.results])
    return _combine(parts)


# revision 6
# speedup vs baseline: 2.5118x; 1.2107x over previous
"""BalanceLoss Trainium2 kernel.

Math restructuring (see reference _balance_loss):
  - pos_gt = (pos_sum >= B/2) and neg_gt are complementary, so maj/min
    groups partition the batch and their counts derive from pos_sum.
  - With y = (1-2t)*x (sign-folded logits, an input encoding choice):
      per-element BCE  = softplus(-x) + (1-t)*x = softplus(y)  exactly,
      easy <=> g < 1/3 <=> softplus(y) < tau,  tau = ln(1.5),
    so ONE softplus and ONE threshold serve both classes.
  - loss needs 5 per-column sums over the batch:
        pos = sum t          T   = sum v         Th  = sum_{hard} v
        S1  = sum_{t=1} v    S1h = sum_{t=1, hard} v      (v = softplus(y))
    then S0 = T - S1, S0h = Th - S1h, and per column:
        loss_c = maj_scale * Shard_maj + min_scale * S_min ;  total /(B*C)

Device (per core, data-parallel over batch), bf16 inputs y, t:
  ACT: q = exp(y) ; v = ln(q + 1) = softplus(y)      (one LUT table)
  DVE: m = [v >= tau] ; hv = m*v ; a1 = t*v ; h1 = t*hv
       (tensor_tensor / tensor_scalar only: these hit the DVE 2x perf mode;
        scalar_tensor_tensor measures 1x so it is avoided)
  Pool: computes the tail half of a1 (the one dependency-free product) to
        keep DVE at ~3.5 passes/chunk, below the ACT wall.
  PE:  ones[128,1]^T @ {t, v, hv, a1, h1} -> 5 PSUM accumulators
Host: unshard + tiny per-column combine (scales, mean).
"""

import numpy as np

B_TOTAL = 131072
C = 128
N_CORES = 8
ROWS = B_TOTAL // N_CORES      # 16384 rows per core
FD = 2048                      # free-dim elements per chunk tile
J = FD // C                    # rows folded per partition per chunk
N_CHUNKS = ROWS * C // (128 * FD)   # 8
MM_N = 512                     # matmul moving free dim (one PSUM bank)
N_STREAMS = 5
SPLIT = 1024                   # a1 columns [SPLIT:] computed on Pool
TAU = float(np.log(1.5))       # softplus(-ln2): easy/hard boundary

_CACHE = {}


def _pin_act_tables():
    """Force the single LUT set containing both exp and ln so the kernel
    loads one ACT table instead of ping-ponging between two (1.3us/reload).
    Set indices must keep matching act_info.json, so empty the others."""
    import concourse.bacc as bacc
    import concourse.hw_specs as hw_specs

    if getattr(hw_specs, "_act_tables_pinned", False):
        return
    orig = hw_specs.get_activation_tables

    def patched(arch):
        tabs = dict(orig(arch))
        keep = "natural_log_exp_and_others"
        if keep in tabs:
            tabs = {n: (s if n == keep else set()) for n, s in tabs.items()}
        return tabs

    hw_specs._act_tables_pinned = True
    hw_specs.get_activation_tables = patched
    bacc.get_activation_tables = patched


def _build_nc():
    import concourse.bacc as bacc
    import concourse.tile as tile
    from concourse import mybir

    _pin_act_tables()

    f32 = mybir.dt.float32
    bf16 = mybir.dt.bfloat16
    AF = mybir.ActivationFunctionType
    OP = mybir.AluOpType

    nc = bacc.Bacc(None)
    yd = nc.dram_tensor("y", [ROWS, C], bf16, kind="ExternalInput")
    td = nc.dram_tensor("t", [ROWS, C], bf16, kind="ExternalInput")
    out = nc.dram_tensor("partials", [1, N_STREAMS * MM_N], f32,
                         kind="ExternalOutput")

    # row = m*FD + p*J + j ; per-partition contiguous J*C elements
    y_r = yd.rearrange("(m p j) c -> m p (j c)", p=128, j=J)
    t_r = td.rearrange("(m p j) c -> m p (j c)", p=128, j=J)

    with tile.TileContext(nc) as tc:
        with (
            tc.tile_pool(name="singles", bufs=1) as singles,
            tc.tile_pool(name="io", bufs=4) as io,
            tc.tile_pool(name="work", bufs=4) as work,
            tc.tile_pool(name="psum", bufs=1, space="PSUM") as psum_pool,
        ):
            ones = singles.tile([128, 1], bf16)
            nc.vector.memset(ones, 1.0)
            acc = psum_pool.tile([1, N_STREAMS * MM_N], f32, tag="acc")
            # Warmup matmul consumes the ones-memset dependency so that
            # steady-state matmuls carry at most one sync wait (walrus
            # LDWEIGHTS codegen supports only one).
            warm = psum_pool.tile([1, 1], f32, tag="warm")
            nc.tensor.matmul(warm, ones, ones, start=True, stop=True)

            def mm(s, mv, lo, hi, m):
                first = m == 0
                last = m == N_CHUNKS - 1
                for jj in range(lo // MM_N, hi // MM_N):
                    nc.tensor.matmul(
                        acc[:, s * MM_N : (s + 1) * MM_N],
                        ones[:, :],
                        mv[:, jj * MM_N : (jj + 1) * MM_N],
                        start=(first and jj == lo // MM_N),
                        stop=(last and jj == hi // MM_N - 1),
                    )

            for m in range(N_CHUNKS):
                y = io.tile([128, FD], bf16, tag="y")
                t = io.tile([128, FD], bf16, tag="t")
                nc.sync.dma_start(y, y_r[m])
                nc.sync.dma_start(t, t_r[m])
                mm(0, t, 0, FD, m)

                q = work.tile([128, FD], bf16, tag="q")
                v = work.tile([128, FD], bf16, tag="v")
                nc.scalar.activation(q, y, AF.Exp)
                nc.scalar.activation(v, q, AF.Ln, bias=1.0)
                mm(1, v, 0, FD, m)

                msk = work.tile([128, FD], bf16, tag="msk")
                hv = work.tile([128, FD], bf16, tag="hv")
                a1 = work.tile([128, FD], bf16, tag="a1")
                h1 = work.tile([128, FD], bf16, tag="h1")
                # hard mask first; a1 between msk and hv hides the DVE
                # write-to-read (RAW) stall on msk
                nc.vector.tensor_scalar(msk, v, TAU, None, OP.is_ge)
                nc.vector.tensor_tensor(a1, t, v, OP.mult)
                mm(3, a1, 0, FD, m)
                nc.vector.tensor_tensor(hv, msk, v, OP.mult)
                mm(2, hv, 0, FD, m)
                nc.vector.tensor_tensor(h1, t, hv, OP.mult)
                mm(4, h1, 0, FD, m)

            res = singles.tile([1, N_STREAMS * MM_N], f32)
            nc.vector.tensor_copy(res, acc)
            nc.sync.dma_start(out[:, :], res)
    nc.finalize()
    return nc


def _get_nc():
    if "nc" not in _CACHE:
        _CACHE["nc"] = _build_nc()
    return _CACHE["nc"]


def _in_maps(pred, target):
    import ml_dtypes

    bf = ml_dtypes.bfloat16
    p32 = np.asarray(pred, dtype=np.float32)
    t32 = np.asarray(target, dtype=np.float32)
    y = ((1.0 - 2.0 * t32) * p32).astype(bf)   # exact sign flip of pred
    t = t32.astype(bf)
    return [
        {
            "y": np.ascontiguousarray(y[i * ROWS : (i + 1) * ROWS]),
            "t": np.ascontiguousarray(t[i * ROWS : (i + 1) * ROWS]),
        }
        for i in range(N_CORES)
    ]


def _combine(parts):
    """parts: [n_cores, 5, MM_N] raw psum rows -> final scalar loss."""
    # psum col q sums j-groups with (j % (MM_N//C)) == q//C at col q % C;
    # fold the leftover j-groups and cores.
    S = parts.reshape(-1, N_STREAMS, MM_N // C, C).sum(axis=(0, 2),
                                                       dtype=np.float64)
    pos, T, Th, s1, s1h = S
    B = float(B_TOTAL)
    s0, s0h = T - s1, Th - s1h
    bal = 0.5 * B
    pos_gt = pos >= bal
    maj_cnt = np.where(pos_gt, pos, B - pos)
    min_cnt = B - maj_cnt
    maj_scale = bal / np.maximum(maj_cnt, 1.0)
    min_scale = np.where(min_cnt > 0, (B - bal) / np.maximum(min_cnt, 1.0), 1.0)
    s_maj_hard = np.where(pos_gt, s1h, s0h)
    s_min = np.where(pos_gt, s0, s1)
    total = (maj_scale * s_maj_hard + min_scale * s_min).sum()
    return np.float32(total / (B * C))


def kernel(pred: np.ndarray, target: np.ndarray) -> np.ndarray:
    from concourse.bass_utils import run_bass_kernel_spmd

    nc = _get_nc()
    res = run_bass_kernel_spmd(
        nc, _in_maps(pred, target), core_ids=list(range(N_CORES)))
    parts = np.stack(
        [r["partials"].reshape(N_STREAMS, MM_N) for r in res.results])
    return _combine(parts)


# revision 12
# speedup vs baseline: 2.5496x; 1.0151x over previous
"""BalanceLoss Trainium2 kernel.

Math restructuring (see reference _balance_loss):
  - pos_gt = (pos_sum >= B/2) and neg_gt are complementary, so maj/min
    groups partition the batch and their counts derive from pos_sum.
  - With y = (1-2t)*x (sign-folded logits, an input encoding choice):
      per-element BCE  = softplus(-x) + (1-t)*x = softplus(y)  exactly,
      easy <=> g < 1/3 <=> softplus(y) < tau,  tau = ln(1.5),
    so ONE softplus and ONE threshold serve both classes.
  - loss needs 5 per-column sums over the batch:
        pos = sum t          T   = sum v         Th  = sum_{hard} v
        S1  = sum_{t=1} v    S1h = sum_{t=1, hard} v      (v = softplus(y))
    then S0 = T - S1, S0h = Th - S1h, and per column:
        loss_c = maj_scale * Shard_maj + min_scale * S_min ;  total /(B*C)

Device (per core, data-parallel over batch), bf16 inputs y, t:
  ACT: q = exp(y) ; v = ln(q + 1) = softplus(y)      (one LUT table)
  DVE: msk = [v >= tau] ; a1 = t*v ; hv = msk*v ; h1 = t*hv
       (tensor_scalar hits the 4x DVE perf mode, tensor_tensor hits 2x;
        scalar_tensor_tensor / fused reduce ops measure 1x so are avoided)
  PE:  ones[128,1]^T @ {t, v, hv, a1, h1} -> 5 PSUM accumulators, emitted
       as one dependency-free burst per chunk so the PE clock stays ramped
Chunks ramp 512->2048->512 to shorten pipeline fill and drain.
Host: unshard + tiny per-column combine (scales, mean).
"""

import numpy as np

B_TOTAL = 131072
C = 128
N_CORES = 8
ROWS = B_TOTAL // N_CORES      # 16384 rows per core
FDMAX = 2048                   # largest free-dim chunk
MM_N = 512                     # matmul moving free dim (one PSUM bank)
N_STREAMS = 5
TAU = float(np.log(1.5))       # softplus(-ln2): easy/hard boundary

# chunk schedule: (j_rows_per_partition, view_index); free = 128*j
# covers rows [off, off+128*j) with off accumulated in order
CHUNK_J = [4, 4, 8, 16, 16, 16, 16, 16, 16, 8, 4, 4]   # sums to 128 j-rows
assert sum(CHUNK_J) * 128 == ROWS

_CACHE = {}


def _pin_act_tables():
    """Force the single LUT set containing both exp and ln so the kernel
    loads one ACT table instead of ping-ponging between two (1.3us/reload).
    Set indices must keep matching act_info.json, so empty the others."""
    import concourse.bacc as bacc
    import concourse.hw_specs as hw_specs

    if getattr(hw_specs, "_act_tables_pinned", False):
        return
    orig = hw_specs.get_activation_tables

    def patched(arch):
        tabs = dict(orig(arch))
        keep = "natural_log_exp_and_others"
        if keep in tabs:
            tabs = {n: (s if n == keep else set()) for n, s in tabs.items()}
        return tabs

    hw_specs._act_tables_pinned = True
    hw_specs.get_activation_tables = patched
    bacc.get_activation_tables = patched


def _build_nc():
    import concourse.bacc as bacc
    import concourse.tile as tile
    from concourse import mybir

    _pin_act_tables()

    f32 = mybir.dt.float32
    bf16 = mybir.dt.bfloat16
    AF = mybir.ActivationFunctionType
    OP = mybir.AluOpType

    nc = bacc.Bacc(None)
    yd = nc.dram_tensor("y", [ROWS, C], bf16, kind="ExternalInput")
    td = nc.dram_tensor("t", [ROWS, C], bf16, kind="ExternalInput")
    out = nc.dram_tensor("partials", [1, N_STREAMS * MM_N], f32,
                         kind="ExternalOutput")

    # chunk m covers rows [off, off+128*j): partition p holds rows
    # off+p*j .. off+(p+1)*j-1, contiguous (j c) in its free dim
    def view(d, off, j):
        return d[off : off + 128 * j].rearrange("(p j) c -> p (j c)", p=128)

    n_chunks = len(CHUNK_J)
    with tile.TileContext(nc) as tc:
        with (
            tc.tile_pool(name="singles", bufs=1) as singles,
            tc.tile_pool(name="io", bufs=4) as io,
            tc.tile_pool(name="work", bufs=4) as work,
            tc.tile_pool(name="psum", bufs=1, space="PSUM") as psum_pool,
        ):
            ones = singles.tile([128, 1], bf16)
            nc.vector.memset(ones, 1.0)
            acc = psum_pool.tile([1, N_STREAMS * MM_N], f32, tag="acc")
            # Warmup matmul consumes the ones-memset dependency so that
            # steady-state matmuls carry at most one sync wait (walrus
            # LDWEIGHTS codegen supports only one).
            warm = psum_pool.tile([1, 1], f32, tag="warm")
            nc.tensor.matmul(warm, ones, ones, start=True, stop=True)

            off = 0
            for m, j in enumerate(CHUNK_J):
                fd = 128 * j
                y = io.tile([128, FDMAX], bf16, tag="y")
                t = io.tile([128, FDMAX], bf16, tag="t")
                nc.sync.dma_start(y[:, 0:fd], view(yd, off, j))
                nc.sync.dma_start(t[:, 0:fd], view(td, off, j))
                off += 128 * j

                q = work.tile([128, FDMAX], bf16, tag="q")
                v = work.tile([128, FDMAX], bf16, tag="v")
                nc.scalar.activation(q[:, 0:fd], y[:, 0:fd], AF.Exp)
                nc.scalar.activation(v[:, 0:fd], q[:, 0:fd], AF.Ln, bias=1.0)

                msk = work.tile([128, FDMAX], bf16, tag="msk")
                hv = work.tile([128, FDMAX], bf16, tag="hv")
                a1 = work.tile([128, FDMAX], bf16, tag="a1")
                h1 = work.tile([128, FDMAX], bf16, tag="h1")
                # msk first; a1 between msk and hv hides the DVE
                # write-to-read (RAW) stall on msk
                nc.vector.tensor_scalar(
                    msk[:, 0:fd], v[:, 0:fd], TAU, None, OP.is_ge)
                nc.vector.tensor_tensor(
                    a1[:, 0:fd], t[:, 0:fd], v[:, 0:fd], OP.mult)
                nc.vector.tensor_tensor(
                    hv[:, 0:fd], msk[:, 0:fd], v[:, 0:fd], OP.mult)
                nc.vector.tensor_tensor(
                    h1[:, 0:fd], t[:, 0:fd], hv[:, 0:fd], OP.mult)

                # one dependency-free matmul burst per chunk keeps the PE
                # clock ramped (no mid-queue semaphore stalls)
                first = m == 0
                last = m == n_chunks - 1
                for s, mv in enumerate([t, v, hv, a1, h1]):
                    for jj in range(fd // MM_N):
                        nc.tensor.matmul(
                            acc[:, s * MM_N : (s + 1) * MM_N],
                            ones[:, :],
                            mv[:, jj * MM_N : (jj + 1) * MM_N],
                            start=(first and jj == 0),
                            stop=(last and jj == fd // MM_N - 1),
                        )

            res = singles.tile([1, N_STREAMS * MM_N], f32)
            # split the PSUM->SBUF drain across DVE and ACT
            nc.vector.tensor_copy(res[:, 0 : 3 * MM_N], acc[:, 0 : 3 * MM_N])
            nc.scalar.copy(
                res[:, 3 * MM_N : 5 * MM_N], acc[:, 3 * MM_N : 5 * MM_N])
            nc.sync.dma_start(out[:, :], res)
    nc.finalize()
    return nc


def _get_nc():
    if "nc" not in _CACHE:
        _CACHE["nc"] = _build_nc()
    return _CACHE["nc"]


def _in_maps(pred, target):
    import ml_dtypes

    bf = ml_dtypes.bfloat16
    p32 = np.asarray(pred, dtype=np.float32)
    t32 = np.asarray(target, dtype=np.float32)
    y = ((1.0 - 2.0 * t32) * p32).astype(bf)   # exact sign flip of pred
    t = t32.astype(bf)
    return [
        {
            "y": np.ascontiguousarray(y[i * ROWS : (i + 1) * ROWS]),
            "t": np.ascontiguousarray(t[i * ROWS : (i + 1) * ROWS]),
        }
        for i in range(N_CORES)
    ]


def _combine(parts):
    """parts: [n_cores, 5, MM_N] raw psum rows -> final scalar loss."""
    # psum col q sums j-groups with (j % (MM_N//C)) == q//C at col q % C;
    # fold the leftover j-groups and cores.
    S = parts.reshape(-1, N_STREAMS, MM_N // C, C).sum(axis=(0, 2),
                                                       dtype=np.float64)
    pos, T, Th, s1, s1h = S
    B = float(B_TOTAL)
    s0, s0h = T - s1, Th - s1h
    bal = 0.5 * B
    pos_gt = pos >= bal
    maj_cnt = np.where(pos_gt, pos, B - pos)
    min_cnt = B - maj_cnt
    maj_scale = bal / np.maximum(maj_cnt, 1.0)
    min_scale = np.where(min_cnt > 0, (B - bal) / np.maximum(min_cnt, 1.0), 1.0)
    s_maj_hard = np.where(pos_gt, s1h, s0h)
    s_min = np.where(pos_gt, s0, s1)
    total = (maj_scale * s_maj_hard + min_scale * s_min).sum()
    return np.float32(total / (B * C))


def kernel(pred: np.ndarray, target: np.ndarray) -> np.ndarray:
    from concourse.bass_utils import run_bass_kernel_spmd

    nc = _get_nc()
    res = run_bass_kernel_spmd(
        nc, _in_maps(pred, target), core_ids=list(range(N_CORES)))
    parts = np.stack(
        [r["partials"].reshape(N_STREAMS, MM_N) for r in res.results])
    return _combine(parts)
